# revision 13
# baseline (speedup 1.0000x reference)
"""Trainium2 Bass kernel for nn_ChannelGroupConvUneven.

Computes, for full inputs
    x      (8, 256, 128, 128) f32
    weight (320, 256, 3, 3)   f32
    bias   (320,)             f32
    param  (5,)               i32   per-group input-channel thresholds
the reference
    out = conv2d(x, weight * mask(param), stride 1, VALID) + bias
    out shape (8, 320, 126, 126) f32
where mask zeroes weight[o, i] for i < param[o // 64].

Strategy: data-parallel over batch — one image per NeuronCore (8 cores).
Weight masking + transposition to the matmul lhsT layout happens on the host
(it is tiny); each core runs a dense 3x3 conv as 18 accumulated matmuls
(2 cin blocks x 9 taps) per output tile in float32r (TF32-like) precision,
accumulating in fp32 PSUM. cout=320 is covered by two full 128-wide matmul
passes plus a 64-wide pass; the 64-wide pass processes two spatial tiles
concurrently in the two column halves of the PE array (tile_position).
"""

import numpy as np

import concourse.mybir as mybir
import concourse.tile as tile
from concourse import bacc
from concourse.bass_utils import run_bass_kernel_spmd


def _ensure_axon_ntff_hook():
    """Best-effort: register the axon NTFF profile hook if the image's
    `antenv` stub lacks `axon_hooks` (concourse's trace path imports it
    unconditionally when BASS_TRACE is set). Purely optional — failures are
    ignored and tracing is simply unavailable."""
    try:
        import sys
        import types

        import antenv

        if "antenv.axon_hooks" in sys.modules:
            return
        mod = types.ModuleType("antenv.axon_hooks")
        _hook = [None]
        mod.set_axon_ntff_profile_hook = lambda h: _hook.__setitem__(0, h)
        mod.get_axon_ntff_profile_hook = lambda: _hook[0]
        sys.modules["antenv.axon_hooks"] = mod
        antenv.axon_hooks = mod
        from trn_agent_boot.trn_boot import _ntff_profile_via_ctypes

        mod.set_axon_ntff_profile_hook(
            _ntff_profile_via_ctypes("/opt/axon/libaxon_pjrt.so")
        )
    except Exception:
        pass


_ensure_axon_ntff_hook()

N_CORES = 8
P = 128
CIN, COUT, KH, KW = 256, 320, 3, 3
H = W = 128
HO = WO = 126
CB = CIN // P  # 2 cin blocks
NTAP = CB * KH * KW  # 18 accumulated matmuls per output tile

# output row tiles: 30 of 4 rows + 2 of 3 rows (N = 504 / 378, both >= 256
# so float32r runs at full rate). Grouped into bands of <= 6 tiles whose
# input rows are DMA'd together (double-buffered).
TILES = [(r, 4) for r in range(0, 120, 4)] + [(120, 3), (123, 3)]
BANDS = [TILES[i : i + 6] for i in range(0, len(TILES), 6)]

CO_FULL = [(0, 128), (128, 128)]  # full-width output-channel blocks
CO_HALF = (256, 64)  # 64-wide block, done as column-tiled pairs

# float32r: PE "fast fp32" mode (TF32-like rounding, fp32 PSUM accumulation),
# 4x the plain-fp32 matmul rate. Measured rel err ~2e-4 vs fp64 reference.
# Set to mybir.dt.float32 for full fp32 (4 cycles/row instead of 1).
MM_DT = mybir.dt.float32r
COL_PAIR = False  # column-tiled pairing for the 64-wide block

_NC_CACHE = {}


def _build_nc(mm_dt, col_pair):
    nc = bacc.Bacc("TRN2", target_bir_lowering=False, debug=False)
    f32 = mybir.dt.float32

    x_d = nc.dram_tensor("x", [CIN, H, W], mm_dt, kind="ExternalInput").ap()
    w_d = nc.dram_tensor(
        "wt", [P, CB, KH, KW, COUT], mm_dt, kind="ExternalInput"
    ).ap()
    b_d = nc.dram_tensor("biasp", [P, 3], f32, kind="ExternalInput").ap()
    o_d = nc.dram_tensor("out", [COUT, HO, WO], f32, kind="ExternalOutput").ap()

    # x viewed as [p, cb, h, w]: cin = cb*128 + p
    x_re = x_d.rearrange("(cb p) h w -> p cb h w", p=P)

    with tile.TileContext(nc) as tc:
        with (
            tc.tile_pool(name="wpool", bufs=1) as wpool,
            tc.tile_pool(name="xpool", bufs=3) as xpool,
            tc.tile_pool(name="opool", bufs=6) as opool,
            tc.tile_pool(name="psum", bufs=8, space="PSUM") as psum_pool,
        ):
            wt = wpool.tile([P, CB, KH, KW, COUT], mm_dt)
            bt = wpool.tile([P, 3], f32)

            def rhs(xb, in_r0, r, rpt, cb, dy, dx):
                rr = r - in_r0 + dy
                return xb[:, cb, rr : rr + rpt, dx : dx + WO]

            for band_idx, band in enumerate(BANDS):
                in_r0 = band[0][0]
                in_rows = band[-1][0] + band[-1][1] + 2 - in_r0
                xb = xpool.tile([P, CB, in_rows, W], mm_dt, tag="xband")
                # Band 0's input rows, the weights, and the bias are split
                # across both HWDGE queues (sync + scalar) and chunked so the
                # first tiles' matmuls start as soon as their slices land
                # (subtile deps). Queue order matters: each queue drains in
                # program order, so the first tile's needs go first. Later
                # bands prefetch on the scalar queue while output stores run
                # on sync.
                if band_idx == 0:
                    for cb in range(CB):
                        eng = nc.sync if cb == 0 else nc.scalar
                        eng.dma_start(
                            xb[:, cb, 0:6], x_re[:, cb, in_r0 : in_r0 + 6, :]
                        )
                    nc.scalar.dma_start(bt[:], b_d[:])
                    for dy in range(KH):
                        nc.sync.dma_start(wt[:, 0, dy], w_d[:, 0, dy])
                        nc.scalar.dma_start(wt[:, 1, dy], w_d[:, 1, dy])
                    for cb in range(CB):
                        eng = nc.sync if cb == 0 else nc.scalar
                        eng.dma_start(
                            xb[:, cb, 6:in_rows],
                            x_re[:, cb, in_r0 + 6 : in_r0 + in_rows, :],
                        )
                else:
                    nc.scalar.dma_start(
                        xb[:], x_re[:, :, in_r0 : in_r0 + in_rows, :]
                    )

                # full-width blocks
                for cob_idx, (co0, com) in enumerate(CO_FULL):
                    for r, rpt in band:
                        ps = psum_pool.tile([P, rpt, WO], f32, tag="ps")
                        k = 0
                        for cb in range(CB):
                            for dy in range(KH):
                                for dx in range(KW):
                                    nc.tensor.matmul(
                                        ps[:com],
                                        wt[:, cb, dy, dx, co0 : co0 + com],
                                        rhs(xb, in_r0, r, rpt, cb, dy, dx),
                                        start=(k == 0),
                                        stop=(k == NTAP - 1),
                                    )
                                    k += 1
                        ot = opool.tile([P, rpt, WO], f32, tag="ot")
                        nc.scalar.add(
                            ot[:com], ps[:com], bt[:com, cob_idx : cob_idx + 1]
                        )
                        nc.sync.dma_start(
                            o_d[co0 : co0 + com, r : r + rpt, :], ot[:com]
                        )

                # 64-wide block
                co0, com = CO_HALF
                if col_pair:
                    for pi in range(0, len(band), 2):
                        (ra, rpta), (rb, rptb) = band[pi], band[pi + 1]
                        assert rpta == rptb
                        ps = psum_pool.tile([P, rpta, WO], f32, tag="ps")
                        k = 0
                        for cb in range(CB):
                            for dy in range(KH):
                                for dx in range(KW):
                                    w_ap = wt[:, cb, dy, dx, co0 : co0 + com]
                                    nc.tensor.matmul(
                                        ps[0:com],
                                        w_ap,
                                        rhs(xb, in_r0, ra, rpta, cb, dy, dx),
                                        start=(k == 0),
                                        stop=False,
                                        tile_position=(0, 0),
                                    )
                                    nc.tensor.matmul(
                                        ps[com : 2 * com],
                                        w_ap,
                                        rhs(xb, in_r0, rb, rptb, cb, dy, dx),
                                        start=False,
                                        stop=(k == NTAP - 1),
                                        tile_position=(0, com),
                                    )
                                    k += 1
                        ot = opool.tile([P, rpta, WO], f32, tag="ot")
                        nc.scalar.add(
                            ot[0:com], ps[0:com], bt[:com, 2:3]
                        )
                        nc.scalar.add(
                            ot[com : 2 * com],
                            ps[com : 2 * com],
                            bt[com : 2 * com, 2:3],
                        )
                        nc.sync.dma_start(
                            o_d[co0 : co0 + com, ra : ra + rpta, :], ot[0:com]
                        )
                        nc.sync.dma_start(
                            o_d[co0 : co0 + com, rb : rb + rptb, :],
                            ot[com : 2 * com],
                        )
                else:
                    for r, rpt in band:
                        ps = psum_pool.tile([P, rpt, WO], f32, tag="ps")
                        k = 0
                        for cb in range(CB):
                            for dy in range(KH):
                                for dx in range(KW):
                                    nc.tensor.matmul(
                                        ps[:com],
                                        wt[:, cb, dy, dx, co0 : co0 + com],
                                        rhs(xb, in_r0, r, rpt, cb, dy, dx),
                                        start=(k == 0),
                                        stop=(k == NTAP - 1),
                                    )
                                    k += 1
                        ot = opool.tile([P, rpt, WO], f32, tag="ot")
                        nc.scalar.add(ot[:com], ps[:com], bt[:com, 2:3])
                        nc.sync.dma_start(
                            o_d[co0 : co0 + com, r : r + rpt, :], ot[:com]
                        )
    nc.compile()
    return nc


def _get_nc():
    key = (str(MM_DT), COL_PAIR)
    if key not in _NC_CACHE:
        _NC_CACHE[key] = _build_nc(MM_DT, COL_PAIR)
    return _NC_CACHE[key]


def _preprocess(x, weight, bias, param):
    x = np.ascontiguousarray(np.asarray(x), dtype=np.float32)
    weight = np.asarray(weight, dtype=np.float32)
    bias = np.asarray(bias, dtype=np.float32)
    param = np.asarray(param)

    # host-side weight masking (group g of 64 output channels uses cin >= param[g])
    thresh = np.repeat(param.astype(np.int64), COUT // param.shape[0])  # [COUT]
    mask = (np.arange(CIN)[None, :] >= thresh[:, None]).astype(np.float32)
    wm = weight * mask[:, :, None, None]
    # lhsT layout: [p, cb, kh, kw, cout]
    wT = np.ascontiguousarray(
        wm.reshape(COUT, CB, P, KH, KW).transpose(2, 1, 3, 4, 0)
    )
    biasp = np.zeros((P, 3), np.float32)
    biasp[:, 0] = bias[0:128]
    biasp[:, 1] = bias[128:256]
    biasp[:64, 2] = bias[256:320]
    # second half of column 2 holds the same 64 biases again, for the
    # column-paired evacuation of partitions 64..127
    biasp[64:, 2] = bias[256:320]
    return x, wT, biasp


def kernel(x, weight, bias, param):
    x, wT, biasp = _preprocess(x, weight, bias, param)
    nc = _get_nc()
    in_maps = [{"x": x[i], "wt": wT, "biasp": biasp} for i in range(N_CORES)]
    res = run_bass_kernel_spmd(nc, in_maps, core_ids=list(range(N_CORES)))
    return np.stack([r["out"] for r in res.results], axis=0)
